# revision 1
# baseline (speedup 1.0000x reference)
"""HELMo encoder (bi-GRU over 3 steps + MHA + classifier) on 8 trn2 cores.

Data-parallel over batch (8192 -> 8 x 1024). Per core, one Bass/Tile kernel:
  A) fused GRU: input and hidden projections accumulate into shared PSUM
     (k = [x; h_prev] against W_cat = [W_ih.T; W_hh.T]), gates on ACT/DVE,
     feature-major layout (features on partitions, batch on free dim).
  B) Q/K/V projections emitted batch-major directly by using hs chunks as the
     matmul stationary operand (out[b, d_out] = hs[d_in, b].T @ W.T[d_in, d_out]).
  C) attention combine on DVE: per-head segment-reduce logits, softmax,
     then ctx_sum = sum_tk (sum_tq w[h,tq,tk]) * V[tk]  (Wo folded over t).
  D) att = ctx_sum @ Wo.T back in feature-major via PE transposes of ctx_sum.
  E) o = att.T @ W_out.T + b_out, softmax over 7 classes.

All big matmuls run in float32r (~1.3e-4 rel err, full PE rate).
"""

import sys

sys.path.insert(0, "/opt/trn_rl_repo")

import numpy as np

import concourse.bacc as bacc
import concourse.bass as bass
import concourse.mybir as mybir
import concourse.tile as tile
from concourse.masks import make_identity

dt = mybir.dt
AF = mybir.ActivationFunctionType
AX = mybir.AxisListType

N_CORES = 8
B = 8192
B_LOC = B // N_CORES          # 1024
I = 1024
H = 1024
D = 2 * H                     # 2048
NH = 16
HD = 128
S = 3
C = 7
P = 128
HJT = H // P                  # 8 jtiles per gate
KC_D = D // P                 # 16

_CACHE = {}


def _r3(ap, pat, **kw):
    return ap.rearrange(pat, **kw)


def build_nc(phases="abcde", reps=1):
    nc = bacc.Bacc("TRN2", target_bir_lowering=False, debug=False,
                   num_devices=N_CORES, dynamic_dma_scratch_size=8192)

    f32, f32r = dt.float32, dt.float32r
    xt = nc.dram_tensor("xt", [S, I, B_LOC], f32r, kind="ExternalInput")
    wcat = {d: nc.dram_tensor(f"wcat_{d}", [2 * H, 3 * H], f32r, kind="ExternalInput")
            for d in ("f", "b")}
    wq = nc.dram_tensor("wq", [D, D], dt.bfloat16, kind="ExternalInput")
    wk = nc.dram_tensor("wk", [D, D], dt.bfloat16, kind="ExternalInput")
    wv = nc.dram_tensor("wv", [D, D], f32r, kind="ExternalInput")
    wo = nc.dram_tensor("wo", [D, D], f32r, kind="ExternalInput")
    wout = nc.dram_tensor("wout", [D, C], f32, kind="ExternalInput")
    brz = {d: nc.dram_tensor(f"brz_{d}", [2 * H, 1], f32, kind="ExternalInput")
           for d in ("f", "b")}
    negbz = {d: nc.dram_tensor(f"negbz_{d}", [H, 1], f32, kind="ExternalInput")
             for d in ("f", "b")}
    bnih = {d: nc.dram_tensor(f"bnih_{d}", [H, 1], f32, kind="ExternalInput")
            for d in ("f", "b")}
    bnhh = {d: nc.dram_tensor(f"bnhh_{d}", [H, 1], f32, kind="ExternalInput")
            for d in ("f", "b")}
    bout = nc.dram_tensor("bout", [1, C], f32, kind="ExternalInput")
    o_out = nc.dram_tensor("o_out", [B_LOC, C], f32, kind="ExternalOutput")
    sm_out = nc.dram_tensor("sm_out", [B_LOC, C], f32, kind="ExternalOutput")

    with tile.TileContext(nc) as tc:
      for _rep in range(reps):
        with tc.tile_pool(name="dram", bufs=1, space="DRAM") as dram:
            hs = dram.tile([S, D, B_LOC], f32r)
            hs_bf = dram.tile([S, D, B_LOC], dt.bfloat16)
            qs = dram.tile([S, B_LOC, D], dt.bfloat16)
            ks = dram.tile([S, B_LOC, D], dt.bfloat16)
            vs = dram.tile([S, B_LOC, D], f32)
            att_d = dram.tile([D, B_LOC], f32)

            # ---------------- Phase A: GRU ----------------
            if "a" in phases:
              with (tc.tile_pool(name="ga_const", bufs=1) as cpool,
                  tc.tile_pool(name="ga_x", bufs=3) as xpool,
                  tc.tile_pool(name="ga_h", bufs=3) as hpool,
                  tc.tile_pool(name="ga_w", bufs=2) as wpool,
                  tc.tile_pool(name="ga_g", bufs=2) as gpool,
                  tc.tile_pool(name="ga_t", bufs=3) as tpool,
                  tc.tile_pool(name="ga_ps", bufs=2, space="PSUM") as pps):
                bias = {}
                for d in ("f", "b"):
                    t_brz = cpool.tile([P, 2 * HJT, 1], f32, tag=f"brz{d}")
                    nc.sync.dma_start(t_brz[:], _r3(brz[d][:], "(c k) o -> k c o", k=P))
                    t_nbz = cpool.tile([P, HJT, 1], f32, tag=f"nbz{d}")
                    nc.sync.dma_start(t_nbz[:], _r3(negbz[d][:], "(c k) o -> k c o", k=P))
                    t_bni = cpool.tile([P, HJT, 1], f32, tag=f"bni{d}")
                    nc.sync.dma_start(t_bni[:], _r3(bnih[d][:], "(c k) o -> k c o", k=P))
                    t_bnh = cpool.tile([P, HJT, 1], f32, tag=f"bnh{d}")
                    nc.sync.dma_start(t_bnh[:], _r3(bnhh[d][:], "(c k) o -> k c o", k=P))
                    bias[d] = (t_brz, t_nbz, t_bni, t_bnh)

                order = [(0, "f", 0), (0, "b", 2), (1, "f", 1),
                         (1, "b", 1), (2, "f", 2), (2, "b", 0)]
                h_cur = {"f": None, "b": None}
                for step, d, t in order:
                    t_brz, t_nbz, t_bni, t_bnh = bias[d]
                    first = step == 0
                    x_halves = []
                    for xh in range(2):
                        xv = xpool.tile([P, HJT // 2, B_LOC], f32r, tag="x",
                                        name=f"x_{step}_{d}_{xh}")
                        nc.sync.dma_start(
                            xv[:],
                            _r3(xt[t][xh * (I // 2):(xh + 1) * (I // 2), :],
                                "(c k) b -> k c b", k=P))
                        x_halves.append(xv)

                    def x_chunk(c):
                        return x_halves[c // (HJT // 2)][:, c % (HJT // 2), :]
                    h_prev = h_cur[d]
                    h_new = hpool.tile([P, HJT, B_LOC], f32r, tag="h")
                    for j in range(HJT):
                        # host pre-permutes wcat columns: per j the r/z/n gate
                        # columns are adjacent -> one contiguous 384-col DMA
                        nkc = HJT if first else 2 * HJT
                        wj = wpool.tile([P, nkc, 3 * P], f32r, tag="wj",
                                        name=f"wj_{step}_{d}_{j}")
                        nc.sync.dma_start(
                            wj[:],
                            _r3(wcat[d][:nkc * P, j * 3 * P:(j + 1) * 3 * P],
                                "(c k) m -> k c m", k=P))
                        wslice = {"wr": wj[:, :, 0:P], "wz": wj[:, :, P:2 * P],
                                  "wn": wj[:, :, 2 * P:3 * P]}
                        for bt in range(2):
                            bs = slice(bt * 512, (bt + 1) * 512)
                            nk = HJT if first else 2 * HJT

                            def mm_acc(ptile, ws):
                                for c in range(nk):
                                    rhs = (x_chunk(c)[:, bs] if c < HJT
                                           else h_prev[:, c - HJT, bs])
                                    nc.tensor.matmul(ptile[:], ws[:, c, :], rhs,
                                                     start=(c == 0),
                                                     stop=(c == nk - 1))

                            pr = pps.tile([P, 512], f32, tag="pr")
                            mm_acc(pr, wslice["wr"])
                            pz = pps.tile([P, 512], f32, tag="pz")
                            mm_acc(pz, wslice["wz"])
                            pgi = pps.tile([P, 512], f32, tag="pgi")
                            for c in range(HJT):
                                nc.tensor.matmul(pgi[:], wslice["wn"][:, c, :],
                                                 x_chunk(c)[:, bs],
                                                 start=(c == 0), stop=(c == HJT - 1))
                            r_sb = gpool.tile([P, 512], f32, tag="r")
                            nc.scalar.activation(r_sb[:], pr[:], AF.Sigmoid,
                                                 bias=t_brz[:, j, :])
                            n_sb = gpool.tile([P, 512], f32, tag="n")
                            if first:
                                zc = gpool.tile([P, 512], f32, tag="z")
                                nc.scalar.activation(zc[:], pz[:], AF.Sigmoid,
                                                     bias=t_nbz[:, j, :], scale=-1.0)
                                nc.scalar.activation(n_sb[:], pgi[:], AF.Tanh,
                                                     bias=t_bni[:, j, :])
                                nc.vector.tensor_mul(h_new[:, j, bs], zc[:], n_sb[:])
                            else:
                                z_sb = gpool.tile([P, 512], f32, tag="z")
                                nc.scalar.activation(z_sb[:], pz[:], AF.Sigmoid,
                                                     bias=t_brz[:, HJT + j, :])
                                pgh = pps.tile([P, 512], f32, tag="pgh")
                                for c in range(HJT, 2 * HJT):
                                    nc.tensor.matmul(pgh[:], wslice["wn"][:, c, :],
                                                     h_prev[:, c - HJT, bs],
                                                     start=(c == HJT),
                                                     stop=(c == 2 * HJT - 1))
                                t1 = tpool.tile([P, 512], f32, tag="tmp")
                                nc.vector.tensor_scalar_add(t1[:], pgh[:],
                                                            t_bnh[:, j, :])
                                t2 = tpool.tile([P, 512], f32, tag="tmp")
                                nc.vector.tensor_mul(t2[:], r_sb[:], t1[:])
                                t3 = tpool.tile([P, 512], f32, tag="tmp")
                                nc.vector.tensor_add(t3[:], pgi[:], t2[:])
                                nc.scalar.activation(n_sb[:], t3[:], AF.Tanh,
                                                     bias=t_bni[:, j, :])
                                t4 = tpool.tile([P, 512], f32, tag="tmp")
                                nc.vector.tensor_sub(t4[:], h_prev[:, j, bs], n_sb[:])
                                t5 = tpool.tile([P, 512], f32, tag="tmp")
                                nc.vector.tensor_mul(t5[:], z_sb[:], t4[:])
                                nc.vector.tensor_add(h_new[:, j, bs], t5[:], n_sb[:])
                            row = (0 if d == "f" else H) + j * P
                            nc.sync.dma_start(hs[t, row:row + P, bs],
                                              h_new[:, j, bs])
                            hb = tpool.tile([P, 512], dt.bfloat16, tag="hbf",
                                            name=f"hbf_{step}_{d}_{j}_{bt}")
                            nc.vector.tensor_copy(hb[:], h_new[:, j, bs])
                            nc.sync.dma_start(hs_bf[t, row:row + P, bs], hb[:])
                    h_cur[d] = h_new

            # ---------------- Phase B: Q/K/V projections ----------------
            if "b" in phases:
              for wsrc, dst, odt, mmdt in (
                      (wq, qs, dt.bfloat16, dt.bfloat16),
                      (wk, ks, dt.bfloat16, dt.bfloat16),
                      (wv, vs, f32, f32r)):
                hsrc = hs_bf if mmdt == dt.bfloat16 else hs
                with (tc.tile_pool(name="gb_w", bufs=1) as wbpool,
                      tc.tile_pool(name="gb_s", bufs=4) as spool,
                      tc.tile_pool(name="gb_o", bufs=3) as opool,
                      tc.tile_pool(name="gb_ps", bufs=8, space="PSUM") as pps):
                    wt = wbpool.tile([P, KC_D, D], mmdt, tag="wproj",
                                     name=f"wproj_{wsrc.name}")
                    nc.sync.dma_start(wt[:], _r3(wsrc[:], "(c k) n -> k c n", k=P))
                    for t in range(S):
                        for btile in range(HJT):
                            hst = spool.tile([P, KC_D, P], mmdt, tag="hst",
                                             name=f"hst_{wsrc.name}_{t}_{btile}")
                            nc.sync.dma_start(
                                hst[:],
                                _r3(hsrc[t][:, btile * P:(btile + 1) * P],
                                    "(c k) b -> k c b", k=P))
                            osb = opool.tile([P, D], odt, tag="qkvout",
                                             name=f"qkvout_{t}_{btile}")
                            for do_ in range(4):
                                po = pps.tile([P, 512], f32, tag="pqkv",
                                              name=f"pqkv_{t}_{btile}_{do_}")
                                for c in range(KC_D):
                                    nc.tensor.matmul(
                                        po[:], hst[:, c, :],
                                        wt[:, c, do_ * 512:(do_ + 1) * 512],
                                        start=(c == 0), stop=(c == KC_D - 1))
                                nc.scalar.copy(osb[:, do_ * 512:(do_ + 1) * 512],
                                               po[:])
                            nc.sync.dma_start(
                                dst[t][btile * P:(btile + 1) * P, :], osb[:])

            # ---------------- Phase C: attention combine ----------------
            if "c" in phases:
              with (tc.tile_pool(name="gc_inqk", bufs=1) as inqk_pool,
                  tc.tile_pool(name="gc_inv", bufs=2) as inv_pool,
                  tc.tile_pool(name="gc_w", bufs=2) as wkpool,
                  tc.tile_pool(name="gc_t", bufs=1) as tmpool,
                  tc.tile_pool(name="gc_c", bufs=2) as ctxpool,
                  tc.tile_pool(name="gc_m", bufs=1) as cm_pool,
                  tc.tile_pool(name="gc_wo", bufs=2) as wopool,
                  tc.tile_pool(name="gc_1", bufs=1) as one_pool,
                  tc.tile_pool(name="gc_ps", bufs=2, space="PSUM") as pps):
                ident = one_pool.tile([P, P], f32, tag="ident")
                make_identity(nc, ident[:])
                ctxm = cm_pool.tile([P, KC_D, B_LOC], f32r, tag="ctxm")
                for btile in range(HJT):
                    bsl = slice(btile * P, (btile + 1) * P)
                    qt, kt, vt = [], [], []
                    for t in range(S):
                        for src_, lst, nm, pool_ in (
                                (qs, qt, "q", inqk_pool), (ks, kt, "k", inqk_pool),
                                (vs, vt, "v", inv_pool)):
                            tl = pool_.tile([P, D],
                                            dt.bfloat16 if nm in ("q", "k") else f32,
                                            tag=f"{nm}{t}",
                                            name=f"{nm}{t}_{btile}")
                            nc.sync.dma_start(tl[:], src_[t][bsl, :])
                            lst.append(tl)
                    L = wkpool.tile([P, NH, S, S], f32, tag="L")
                    prod_tag = 0
                    for tq in range(S):
                        for tk in range(S):
                            pr_ = wkpool.tile([P, D], dt.bfloat16, tag="prod",
                                              name=f"prod_{btile}_{tq}_{tk}")
                            nc.vector.tensor_mul(pr_[:], qt[tq][:], kt[tk][:])
                            nc.vector.reduce_sum(
                                L[:, :, tq, tk],
                                _r3(pr_[:], "p (h e) -> p h e", h=NH), axis=AX.X)
                            prod_tag += 1
                    # logits are O(1e-3): exp cannot overflow, skip max-sub
                    E2 = wkpool.tile([P, NH, S, S], f32, tag="E2")
                    nc.scalar.activation(E2[:], L[:], AF.Exp)
                    Ssum = wkpool.tile([P, NH, S], f32, tag="Ssum")
                    nc.vector.reduce_sum(Ssum[:], E2[:], axis=AX.X)
                    Rs = wkpool.tile([P, NH, S], f32, tag="Rs")
                    nc.vector.reciprocal(Rs[:], Ssum[:])
                    Wn = wkpool.tile([P, NH, S, S], f32, tag="Wn")
                    nc.vector.tensor_mul(Wn[:], E2[:],
                                         Rs[:, :, :, None].broadcast_to([P, NH, S, S]))
                    wsum = wkpool.tile([P, NH, S], f32, tag="wsum")
                    nc.vector.reduce_sum(wsum[:], _r3(Wn[:], "p h q k -> p h k q"),
                                         axis=AX.X)
                    ctx = ctxpool.tile([P, D], f32, tag="ctx")
                    tm0 = tmpool.tile([P, D], f32, tag="ctmp0")
                    nc.vector.tensor_mul(
                        _r3(tm0[:], "p (h e) -> p h e", h=NH),
                        _r3(vt[0][:], "p (h e) -> p h e", h=NH),
                        wsum[:, :, 0][:, :, None].broadcast_to([P, NH, HD]))
                    tm1 = tmpool.tile([P, D], f32, tag="ctmp1")
                    nc.vector.tensor_mul(
                        _r3(tm1[:], "p (h e) -> p h e", h=NH),
                        _r3(vt[1][:], "p (h e) -> p h e", h=NH),
                        wsum[:, :, 1][:, :, None].broadcast_to([P, NH, HD]))
                    nc.vector.tensor_add(tm0[:], tm0[:], tm1[:])
                    nc.vector.tensor_mul(
                        _r3(tm1[:], "p (h e) -> p h e", h=NH),
                        _r3(vt[2][:], "p (h e) -> p h e", h=NH),
                        wsum[:, :, 2][:, :, None].broadcast_to([P, NH, HD]))
                    nc.vector.tensor_add(ctx[:], tm0[:], tm1[:])
                    for c in range(KC_D):
                        pt = pps.tile([P, P], f32, tag="ptr")
                        nc.tensor.transpose(pt[:], ctx[:, c * P:(c + 1) * P],
                                            ident[:])
                        nc.vector.tensor_copy(ctxm[:, c, bsl], pt[:])
                    # after each half of the btiles, run the Wo half-pass on PE
                    # so it overlaps the DVE combine of the remaining btiles
                    if btile in (3, 7):
                        bt = btile // 4
                        bs = slice(bt * 512, (bt + 1) * 512)
                        for jt in range(KC_D):
                            wos = wopool.tile([P, KC_D, P], f32r, tag="wos",
                                              name=f"wos_{bt}_{jt}")
                            nc.sync.dma_start(
                                wos[:],
                                _r3(wo[:, jt * P:(jt + 1) * P],
                                    "(c k) m -> k c m", k=P))
                            pw = pps.tile([P, 512], f32, tag="pwo",
                                          name=f"pwo_{bt}_{jt}")
                            for c in range(KC_D):
                                nc.tensor.matmul(pw[:], wos[:, c, :],
                                                 ctxm[:, c, bs],
                                                 start=(c == 0),
                                                 stop=(c == KC_D - 1))
                            asb = ctxpool.tile([P, 512], f32, tag="asb",
                                               name=f"asb_{bt}_{jt}")
                            nc.vector.tensor_copy(asb[:], pw[:])
                            nc.sync.dma_start(att_d[jt * P:(jt + 1) * P, bs],
                                              asb[:])

            # ---------------- Phase E: classifier + softmax ----------------
            if "d" in phases:
                with (tc.tile_pool(name="ge", bufs=2) as epool,
                      tc.tile_pool(name="ge1", bufs=1) as e1pool,
                      tc.tile_pool(name="ge_ps", bufs=2, space="PSUM") as pps2):
                    wout_sb = e1pool.tile([P, KC_D, C], f32, tag="wout")
                    nc.sync.dma_start(wout_sb[:], _r3(wout[:], "(c k) n -> k c n", k=P))
                    bout_sb = e1pool.tile([P, C], f32, tag="bout")
                    nc.sync.dma_start(bout_sb[:], bout[:].to_broadcast([P, C]))
                    for btile in range(HJT):
                        bsl = slice(btile * P, (btile + 1) * P)
                        attt = epool.tile([P, KC_D, P], f32, tag="attt",
                                          name=f"attt_{btile}")
                        nc.sync.dma_start(attt[:], _r3(att_d[:, bsl],
                                                       "(c k) b -> k c b", k=P))
                        pf = pps2.tile([P, C], f32, tag="pf")
                        for c in range(KC_D):
                            nc.tensor.matmul(pf[:], attt[:, c, :],
                                             wout_sb[:, c, :],
                                             start=(c == 0), stop=(c == KC_D - 1))
                        o_sb = epool.tile([P, C], f32, tag="osb")
                        nc.vector.tensor_add(o_sb[:], pf[:], bout_sb[:])
                        nc.sync.dma_start(o_out[bsl, :], o_sb[:])
                        mx = epool.tile([P, 1], f32, tag="mx")
                        nc.vector.reduce_max(mx[:], o_sb[:], axis=AX.X)
                        nmx = epool.tile([P, 1], f32, tag="nmx")
                        nc.vector.tensor_scalar_mul(nmx[:], mx[:], -1.0)
                        esb = epool.tile([P, C], f32, tag="esb")
                        nc.scalar.activation(esb[:], o_sb[:], AF.Exp, bias=nmx[:])
                        ssb = epool.tile([P, 1], f32, tag="ssb")
                        nc.vector.reduce_sum(ssb[:], esb[:], axis=AX.X)
                        rsb = epool.tile([P, 1], f32, tag="rsb")
                        nc.vector.reciprocal(rsb[:], ssb[:])
                        smsb = epool.tile([P, C], f32, tag="smsb")
                        nc.vector.tensor_mul(smsb[:], esb[:],
                                             rsb[:].broadcast_to([P, C]))
                        nc.sync.dma_start(sm_out[bsl, :], smsb[:])

    nc.compile()
    return nc


def _prep_inputs(inputs):
    f32 = np.float32
    xs = np.stack([np.asarray(inputs["x1"], f32), np.asarray(inputs["x2"], f32),
                   np.asarray(inputs["x3"], f32)])  # (3, B, I)
    shared = {}
    for d in ("f", "b"):
        wih = np.asarray(inputs[f"W_ih_{d}"], f32)
        whh = np.asarray(inputs[f"W_hh_{d}"], f32)
        bih = np.asarray(inputs[f"b_ih_{d}"], f32)
        bhh = np.asarray(inputs[f"b_hh_{d}"], f32)
        wc = np.concatenate([wih.T, whh.T], axis=0)  # (2I, 3H)
        cols = []
        for j in range(HJT):
            for g in range(3):
                cols.append(wc[:, (g * H + j * P):(g * H + (j + 1) * P)])
        shared[f"wcat_{d}"] = np.ascontiguousarray(np.concatenate(cols, axis=1))
        bsum = bih + bhh
        shared[f"brz_{d}"] = np.ascontiguousarray(bsum[:2 * H, None])
        shared[f"negbz_{d}"] = np.ascontiguousarray(-bsum[H:2 * H, None])
        shared[f"bnih_{d}"] = np.ascontiguousarray(bih[2 * H:, None])
        shared[f"bnhh_{d}"] = np.ascontiguousarray(bhh[2 * H:, None])
    import ml_dtypes
    shared["wq"] = np.ascontiguousarray(
        (np.asarray(inputs["Wq"], f32).T * (HD ** -0.5)).astype(ml_dtypes.bfloat16))
    shared["wk"] = np.ascontiguousarray(
        np.asarray(inputs["Wk"], f32).T.astype(ml_dtypes.bfloat16))
    shared["wv"] = np.ascontiguousarray(np.asarray(inputs["Wv"], f32).T)
    shared["wo"] = np.ascontiguousarray(np.asarray(inputs["Wo"], f32).T)
    shared["wout"] = np.ascontiguousarray(np.asarray(inputs["W_out"], f32).T)
    shared["bout"] = np.ascontiguousarray(np.asarray(inputs["b_out"], f32)[None, :])

    in_maps = []
    for c in range(N_CORES):
        rows = slice(c * B_LOC, (c + 1) * B_LOC)
        m = dict(shared)
        m["xt"] = np.ascontiguousarray(xs[:, rows, :].transpose(0, 2, 1))
        in_maps.append(m)
    return in_maps


def _get_nc():
    if "nc" not in _CACHE:
        _CACHE["nc"] = build_nc()
    return _CACHE["nc"]


def kernel(**inputs):
    from concourse.bass_utils import run_bass_kernel_spmd

    nc = _get_nc()
    in_maps = _prep_inputs(inputs)
    res = run_bass_kernel_spmd(nc, in_maps, core_ids=list(range(N_CORES)))
    o = np.concatenate([res.results[c]["o_out"] for c in range(N_CORES)], axis=0)
    sm = np.concatenate([res.results[c]["sm_out"] for c in range(N_CORES)], axis=0)
    return o, sm



# revision 6
# speedup vs baseline: 1.0892x; 1.0892x over previous
"""HELMo encoder (bi-GRU over 3 steps + MHA + classifier) on 8 trn2 cores.

Data-parallel over batch (8192 -> 8 x 1024). Per core, one Bass/Tile kernel:
  A) fused GRU in bf16: input+hidden projections accumulate into f32 PSUM
     (k = [x; h_prev] vs W_cat = [W_ih.T; W_hh.T]); gates split across
     ACT (sigmoid/tanh), DVE (psum-side adds/muls) and Pool (SBUF-side
     combine); h stored bf16, feature-major.
  B) fused attention pass, one 128-row batch chunk at a time, all-SBUF:
     - Q/K projections as fp8e4 DoubleRow matmuls (2x PE rate); weights
       pre-scaled x16, hs x4 (exact powers of 2, fp8 normal range), with
       the compensation folded into the softmax exp scale.
     - V projection in bf16, W_v streamed per chunk in 512-col pieces.
     - per-head logits / softmax / (sum_tq w) combine on DVE+Pool.
     - W_o is folded into the classifier on the host
       (W_eff = W_out @ W_o), so ctx_sum goes straight to the 7-class
       head after a PE transpose; softmax on ACT/DVE.

Matmul accumulation is f32 PSUM throughout; rel err vs f32 reference
~2e-3, dominated by bf16 weight quantization.
"""

import sys

sys.path.insert(0, "/opt/trn_rl_repo")

import numpy as np

import concourse.bacc as bacc
import concourse.bass as bass
import concourse.mybir as mybir
import concourse.tile as tile
from concourse.masks import make_identity

dt = mybir.dt
AF = mybir.ActivationFunctionType
AX = mybir.AxisListType
PM = mybir.MatmulPerfMode

N_CORES = 8
B = 8192
B_LOC = B // N_CORES          # 1024
I = 1024
H = 1024
D = 2 * H                     # 2048
NH = 16
HD = 128
S = 3
C = 7
P = 128
HJT = H // P                  # 8 jtiles per gate
KC_D = D // P                 # 16
NCH = B_LOC // P              # 8 batch chunks in the attention pass

W8_SCALE = 16.0               # fp8 weight pre-scale (host side)
H8_SCALE = 4.0                # fp8 activation pre-scale (device side)
EXP_SCALE = 1.0 / (W8_SCALE * W8_SCALE * H8_SCALE * H8_SCALE * HD ** 0.5)
USE_DOUBLE_ROW = True

f32, bf, f8 = dt.float32, dt.bfloat16, dt.float8e4

_CACHE = {}


def _r3(ap, pat, **kw):
    return ap.rearrange(pat, **kw)


def _gru_cell(nc, pps, gpool, tpool, bias_t, j, bs, first,
              x_chunk, h_prev, h_new, wr, wz, wn):
    t_brz, t_nbz, t_bni, t_bnh = bias_t
    if first:
        pz = pps.tile([P, 512], f32, tag="pz")
        for c in range(HJT):
            nc.tensor.matmul(pz[:], wz[:, c, :], x_chunk(c)[:, bs],
                             start=(c == 0), stop=(c == HJT - 1))
        pgi = pps.tile([P, 512], f32, tag="pgi")
        for c in range(HJT):
            nc.tensor.matmul(pgi[:], wn[:, c, :], x_chunk(c)[:, bs],
                             start=(c == 0), stop=(c == HJT - 1))
        zc = gpool.tile([P, 512], f32, tag="z")
        nc.scalar.activation(zc[:], pz[:], AF.Sigmoid,
                             bias=t_nbz[:, j, :], scale=-1.0)
        n_sb = gpool.tile([P, 512], f32, tag="n")
        nc.scalar.activation(n_sb[:], pgi[:], AF.Tanh, bias=t_bni[:, j, :])
        nc.gpsimd.tensor_mul(h_new[:, j, bs], zc[:], n_sb[:])
        return

    nk = 2 * HJT

    def mm_acc(ptile, ws):
        for c in range(nk):
            rhs = (x_chunk(c)[:, bs] if c < HJT else h_prev[:, c - HJT, bs])
            nc.tensor.matmul(ptile[:], ws[:, c, :], rhs,
                             start=(c == 0), stop=(c == nk - 1))

    pr = pps.tile([P, 512], f32, tag="pr")
    mm_acc(pr, wr)
    pz = pps.tile([P, 512], f32, tag="pz")
    mm_acc(pz, wz)
    pgi = pps.tile([P, 512], f32, tag="pgi")
    for c in range(HJT):
        nc.tensor.matmul(pgi[:], wn[:, c, :], x_chunk(c)[:, bs],
                         start=(c == 0), stop=(c == HJT - 1))
    pgh = pps.tile([P, 512], f32, tag="pgh")
    for c in range(HJT, 2 * HJT):
        nc.tensor.matmul(pgh[:], wn[:, c, :], h_prev[:, c - HJT, bs],
                         start=(c == HJT), stop=(c == 2 * HJT - 1))
    r_sb = gpool.tile([P, 512], f32, tag="r")
    nc.scalar.activation(r_sb[:], pr[:], AF.Sigmoid, bias=t_brz[:, j, :])
    z_sb = gpool.tile([P, 512], f32, tag="z")
    nc.scalar.activation(z_sb[:], pz[:], AF.Sigmoid, bias=t_brz[:, HJT + j, :])
    t1 = tpool.tile([P, 512], f32, tag="tmp")
    nc.vector.tensor_scalar_add(t1[:], pgh[:], t_bnh[:, j, :])
    t2 = tpool.tile([P, 512], f32, tag="tmp")
    nc.vector.tensor_mul(t2[:], r_sb[:], t1[:])
    t3 = tpool.tile([P, 512], f32, tag="tmp")
    nc.vector.tensor_add(t3[:], pgi[:], t2[:])
    n_sb = gpool.tile([P, 512], f32, tag="n")
    nc.scalar.activation(n_sb[:], t3[:], AF.Tanh, bias=t_bni[:, j, :])
    t4 = tpool.tile([P, 512], bf, tag="tmpb")
    nc.gpsimd.tensor_sub(t4[:], h_prev[:, j, bs], n_sb[:])
    t5 = tpool.tile([P, 512], bf, tag="tmpb")
    nc.gpsimd.tensor_mul(t5[:], z_sb[:], t4[:])
    nc.gpsimd.tensor_add(h_new[:, j, bs], t5[:], n_sb[:])


def _phase_a(nc, tc, T, hs_bf):
    with (tc.tile_pool(name="ga_const", bufs=1) as cpool,
          tc.tile_pool(name="ga_x", bufs=3) as xpool,
          tc.tile_pool(name="ga_h", bufs=3) as hpool,
          tc.tile_pool(name="ga_w", bufs=2) as wpool,
          tc.tile_pool(name="ga_g", bufs=2) as gpool,
          tc.tile_pool(name="ga_t", bufs=3) as tpool,
          tc.tile_pool(name="ga_ps", bufs=2, space="PSUM") as pps):
        bias = {}
        for d in ("f", "b"):
            t_brz = cpool.tile([P, 2 * HJT, 1], f32, tag=f"brz{d}")
            nc.sync.dma_start(t_brz[:], _r3(T[f"brz_{d}"][:], "(c k) o -> k c o", k=P))
            t_nbz = cpool.tile([P, HJT, 1], f32, tag=f"nbz{d}")
            nc.sync.dma_start(t_nbz[:], _r3(T[f"negbz_{d}"][:], "(c k) o -> k c o", k=P))
            t_bni = cpool.tile([P, HJT, 1], f32, tag=f"bni{d}")
            nc.sync.dma_start(t_bni[:], _r3(T[f"bnih_{d}"][:], "(c k) o -> k c o", k=P))
            t_bnh = cpool.tile([P, HJT, 1], f32, tag=f"bnh{d}")
            nc.sync.dma_start(t_bnh[:], _r3(T[f"bnhh_{d}"][:], "(c k) o -> k c o", k=P))
            bias[d] = (t_brz, t_nbz, t_bni, t_bnh)

        order = [(0, "f", 0), (0, "b", 2), (1, "f", 1),
                 (1, "b", 1), (2, "f", 2), (2, "b", 0)]
        h_cur = {"f": None, "b": None}
        for step, d, t in order:
            first = step == 0
            x_halves = []
            for xh in range(2):
                xv = xpool.tile([P, HJT // 2, B_LOC], bf, tag="x",
                                name=f"x_{step}_{d}_{xh}")
                nc.sync.dma_start(
                    xv[:],
                    _r3(T["xt"][t][xh * (I // 2):(xh + 1) * (I // 2), :],
                        "(c k) b -> k c b", k=P))
                x_halves.append(xv)

            def x_chunk(c):
                return x_halves[c // (HJT // 2)][:, c % (HJT // 2), :]

            h_prev = h_cur[d]
            h_new = hpool.tile([P, HJT, B_LOC], bf, tag="h",
                               name=f"h_{step}_{d}")
            for j in range(HJT):
                # host pre-permutes wcat columns: per j the r/z/n gate
                # columns are adjacent -> one contiguous 384-col DMA
                nkc = HJT if first else 2 * HJT
                wj = wpool.tile([P, nkc, 3 * P], bf, tag="wj",
                                name=f"wj_{step}_{d}_{j}")
                nc.sync.dma_start(
                    wj[:],
                    _r3(T[f"wcat_{d}"][:nkc * P, j * 3 * P:(j + 1) * 3 * P],
                        "(c k) m -> k c m", k=P))
                wr = wj[:, :, 0:P]
                wz = wj[:, :, P:2 * P]
                wn = wj[:, :, 2 * P:3 * P]
                for bt in range(2):
                    bs = slice(bt * 512, (bt + 1) * 512)
                    _gru_cell(nc, pps, gpool, tpool, bias[d], j, bs, first,
                              x_chunk, h_prev, h_new, wr, wz, wn)
                    row = (0 if d == "f" else H) + j * P
                    nc.sync.dma_start(hs_bf[t, row:row + P, bs],
                                      h_new[:, j, bs])
            h_cur[d] = h_new


def _attn_chunk(nc, pools, consts, T, hs_bf, ci):
    (hpool, h8pool, wvpool, qkpool, vpool, appool, smpool, ctxpool, cmpool,
     epool, pps_qk, pps_v, pps_t, pps_c) = pools
    wq8_sb, wk8_sb, weff_sb, bout_sb, ident = consts
    bsl = slice(ci * P, (ci + 1) * P)
    hst, hst8 = [], []
    for t in range(S):
        ht = hpool.tile([P, KC_D, P], bf, tag=f"hst{t}", name=f"hst{t}_{ci}")
        nc.sync.dma_start(ht[:], _r3(hs_bf[t][:, bsl], "(c k) b -> k c b", k=P))
        h8 = h8pool.tile([P, KC_D, P], f8, tag=f"h8_{t}", name=f"h8_{t}_{ci}")
        nc.gpsimd.tensor_scalar_mul(h8[:], ht[:], H8_SCALE)
        hst.append(ht)
        hst8.append(h8)

    qt, kt = [], []
    for t in range(S):
        for wsb, lst, nm in ((wq8_sb, qt, "q"), (wk8_sb, kt, "k")):
            osb = qkpool.tile([P, D], bf, tag=f"{nm}{t}", name=f"{nm}{t}_{ci}")
            for do_ in range(4):
                po = pps_qk.tile([P, 512], f32, tag="pqk",
                                 name=f"p{nm}_{t}_{ci}_{do_}")
                if USE_DOUBLE_ROW:
                    for c in range(0, KC_D, 2):
                        nc.tensor.matmul(
                            po[:], hst8[t][:, c:c + 2, :],
                            wsb[:, c:c + 2, do_ * 512:(do_ + 1) * 512],
                            start=(c == 0), stop=(c == KC_D - 2),
                            perf_mode=PM.DoubleRow)
                else:
                    for c in range(KC_D):
                        nc.tensor.matmul(
                            po[:], hst8[t][:, c, :],
                            wsb[:, c, do_ * 512:(do_ + 1) * 512],
                            start=(c == 0), stop=(c == KC_D - 1))
                dsl = slice(do_ * 512, (do_ + 1) * 512)
                if nm == "q":
                    nc.scalar.copy(osb[:, dsl], po[:])
                else:
                    nc.vector.tensor_copy(osb[:, dsl], po[:])
            lst.append(osb)

    vt = [vpool.tile([P, D], bf, tag=f"v{t}", name=f"v{t}_{ci}")
          for t in range(S)]
    for do_ in range(4):
        wvt = wvpool.tile([P, KC_D, 512], bf, tag="wv", name=f"wv_{ci}_{do_}")
        nc.sync.dma_start(
            wvt[:],
            _r3(T["wv"][:, do_ * 512:(do_ + 1) * 512], "(c k) m -> k c m", k=P))
        for t in range(S):
            pv = pps_v.tile([P, 512], f32, tag="pv", name=f"pv_{ci}_{do_}_{t}")
            for c in range(KC_D):
                nc.tensor.matmul(pv[:], hst[t][:, c, :], wvt[:, c, :],
                                 start=(c == 0), stop=(c == KC_D - 1))
            dsl = slice(do_ * 512, (do_ + 1) * 512)
            if (do_ + t) % 2 == 0:
                nc.scalar.copy(vt[t][:, dsl], pv[:])
            else:
                nc.vector.tensor_copy(vt[t][:, dsl], pv[:])

    L = smpool.tile([P, NH, S, S], f32, tag="L")
    for tq in range(S):
        for tk in range(S):
            pr_ = appool.tile([P, D], bf, tag="prod", name=f"prod_{ci}_{tq}_{tk}")
            nc.gpsimd.tensor_mul(pr_[:], qt[tq][:], kt[tk][:])
            nc.vector.reduce_sum(
                L[:, :, tq, tk], _r3(pr_[:], "p (h e) -> p h e", h=NH), axis=AX.X)
    # logits are O(0.1) after descaling: exp cannot overflow, skip max-sub
    E2 = smpool.tile([P, NH, S, S], f32, tag="E2")
    nc.scalar.activation(E2[:], L[:], AF.Exp, scale=EXP_SCALE)
    Ssum = smpool.tile([P, NH, S], f32, tag="Ssum")
    nc.vector.reduce_sum(Ssum[:], E2[:], axis=AX.X)
    Rs = smpool.tile([P, NH, S], f32, tag="Rs")
    nc.vector.reciprocal(Rs[:], Ssum[:])
    Wn = smpool.tile([P, NH, S, S], f32, tag="Wn")
    nc.vector.tensor_mul(Wn[:], E2[:],
                         Rs[:, :, :, None].broadcast_to([P, NH, S, S]))
    wsum = smpool.tile([P, NH, S], f32, tag="wsum")
    nc.vector.reduce_sum(wsum[:], _r3(Wn[:], "p h q k -> p h k q"), axis=AX.X)

    tm0 = ctxpool.tile([P, D], bf, tag="tm0", name=f"tm0_{ci}")
    nc.gpsimd.tensor_mul(
        _r3(tm0[:], "p (h e) -> p h e", h=NH),
        _r3(vt[0][:], "p (h e) -> p h e", h=NH),
        wsum[:, :, 0][:, :, None].broadcast_to([P, NH, HD]))
    tm1 = ctxpool.tile([P, D], bf, tag="tm1", name=f"tm1_{ci}")
    nc.vector.tensor_mul(
        _r3(tm1[:], "p (h e) -> p h e", h=NH),
        _r3(vt[1][:], "p (h e) -> p h e", h=NH),
        wsum[:, :, 1][:, :, None].broadcast_to([P, NH, HD]))
    nc.gpsimd.tensor_add(tm0[:], tm0[:], tm1[:])
    tm2 = ctxpool.tile([P, D], bf, tag="tm2", name=f"tm2_{ci}")
    nc.vector.tensor_mul(
        _r3(tm2[:], "p (h e) -> p h e", h=NH),
        _r3(vt[2][:], "p (h e) -> p h e", h=NH),
        wsum[:, :, 2][:, :, None].broadcast_to([P, NH, HD]))
    ctx = ctxpool.tile([P, D], bf, tag="ctx", name=f"ctx_{ci}")
    nc.gpsimd.tensor_add(ctx[:], tm0[:], tm2[:])

    ctxm = cmpool.tile([P, KC_D, P], bf, tag="ctxm", name=f"ctxm_{ci}")
    for c in range(KC_D):
        pt = pps_t.tile([P, P], bf, tag="pt", name=f"pt_{ci}_{c}")
        nc.tensor.transpose(pt[:], ctx[:, c * P:(c + 1) * P], ident[:])
        nc.vector.tensor_copy(ctxm[:, c, :], pt[:])
    pf = pps_c.tile([P, C], f32, tag="pf", name=f"pf_{ci}")
    for c in range(KC_D):
        nc.tensor.matmul(pf[:], ctxm[:, c, :], weff_sb[:, c, :],
                         start=(c == 0), stop=(c == KC_D - 1))
    o_sb = epool.tile([P, C], f32, tag="osb", name=f"osb_{ci}")
    nc.vector.tensor_add(o_sb[:], pf[:], bout_sb[:])
    nc.sync.dma_start(T["o_out"][bsl, :], o_sb[:])
    mx = epool.tile([P, 1], f32, tag="mx")
    nc.vector.reduce_max(mx[:], o_sb[:], axis=AX.X)
    nmx = epool.tile([P, 1], f32, tag="nmx")
    nc.vector.tensor_scalar_mul(nmx[:], mx[:], -1.0)
    esb = epool.tile([P, C], f32, tag="esb")
    nc.scalar.activation(esb[:], o_sb[:], AF.Exp, bias=nmx[:])
    ssb = epool.tile([P, 1], f32, tag="ssb")
    nc.vector.reduce_sum(ssb[:], esb[:], axis=AX.X)
    rsb = epool.tile([P, 1], f32, tag="rsb")
    nc.vector.reciprocal(rsb[:], ssb[:])
    smsb = epool.tile([P, C], f32, tag="smsb")
    nc.vector.tensor_mul(smsb[:], esb[:], rsb[:].broadcast_to([P, C]))
    nc.sync.dma_start(T["sm_out"][bsl, :], smsb[:])


def _phase_b(nc, tc, T, hs_bf):
    with (tc.tile_pool(name="gb_c", bufs=1) as cpool,
          tc.tile_pool(name="gb_h", bufs=2) as hpool,
          tc.tile_pool(name="gb_h8", bufs=2) as h8pool,
          tc.tile_pool(name="gb_wv", bufs=2) as wvpool,
          tc.tile_pool(name="gb_qk", bufs=1) as qkpool,
          tc.tile_pool(name="gb_v", bufs=1) as vpool,
          tc.tile_pool(name="gb_ap", bufs=2) as appool,
          tc.tile_pool(name="gb_sm", bufs=1) as smpool,
          tc.tile_pool(name="gb_ctx", bufs=1) as ctxpool,
          tc.tile_pool(name="gb_cm", bufs=2) as cmpool,
          tc.tile_pool(name="gb_e", bufs=2) as epool,
          tc.tile_pool(name="gb_pqk", bufs=4, space="PSUM") as pps_qk,
          tc.tile_pool(name="gb_pv", bufs=2, space="PSUM") as pps_v,
          tc.tile_pool(name="gb_pt", bufs=1, space="PSUM") as pps_t,
          tc.tile_pool(name="gb_pc", bufs=1, space="PSUM") as pps_c):
        wq8_sb = cpool.tile([P, KC_D, D], f8, tag="wq8")
        nc.sync.dma_start(wq8_sb[:], _r3(T["wq8"][:], "(c k) n -> k c n", k=P))
        wk8_sb = cpool.tile([P, KC_D, D], f8, tag="wk8")
        nc.sync.dma_start(wk8_sb[:], _r3(T["wk8"][:], "(c k) n -> k c n", k=P))
        weff_sb = cpool.tile([P, KC_D, C], bf, tag="weff")
        nc.sync.dma_start(weff_sb[:], _r3(T["weff"][:], "(c k) n -> k c n", k=P))
        bout_sb = cpool.tile([P, C], f32, tag="bout")
        nc.sync.dma_start(bout_sb[:], T["bout"][:].to_broadcast([P, C]))
        ident = cpool.tile([P, P], bf, tag="ident")
        make_identity(nc, ident[:])

        pools = (hpool, h8pool, wvpool, qkpool, vpool, appool, smpool,
                 ctxpool, cmpool, epool, pps_qk, pps_v, pps_t, pps_c)
        consts = (wq8_sb, wk8_sb, weff_sb, bout_sb, ident)
        for ci in range(NCH):
            _attn_chunk(nc, pools, consts, T, hs_bf, ci)


def build_nc(reps=1):
    nc = bacc.Bacc("TRN2", target_bir_lowering=False, debug=False,
                   num_devices=N_CORES, dynamic_dma_scratch_size=8192)

    T = {}
    T["xt"] = nc.dram_tensor("xt", [S, I, B_LOC], bf, kind="ExternalInput")
    for d in ("f", "b"):
        T[f"wcat_{d}"] = nc.dram_tensor(f"wcat_{d}", [2 * H, 3 * H], bf,
                                        kind="ExternalInput")
        T[f"brz_{d}"] = nc.dram_tensor(f"brz_{d}", [2 * H, 1], f32,
                                       kind="ExternalInput")
        T[f"negbz_{d}"] = nc.dram_tensor(f"negbz_{d}", [H, 1], f32,
                                         kind="ExternalInput")
        T[f"bnih_{d}"] = nc.dram_tensor(f"bnih_{d}", [H, 1], f32,
                                        kind="ExternalInput")
        T[f"bnhh_{d}"] = nc.dram_tensor(f"bnhh_{d}", [H, 1], f32,
                                        kind="ExternalInput")
    T["wq8"] = nc.dram_tensor("wq8", [D, D], f8, kind="ExternalInput")
    T["wk8"] = nc.dram_tensor("wk8", [D, D], f8, kind="ExternalInput")
    T["wv"] = nc.dram_tensor("wv", [D, D], bf, kind="ExternalInput")
    T["weff"] = nc.dram_tensor("weff", [D, C], bf, kind="ExternalInput")
    T["bout"] = nc.dram_tensor("bout", [1, C], f32, kind="ExternalInput")
    T["o_out"] = nc.dram_tensor("o_out", [B_LOC, C], f32, kind="ExternalOutput")
    T["sm_out"] = nc.dram_tensor("sm_out", [B_LOC, C], f32,
                                 kind="ExternalOutput")

    with tile.TileContext(nc) as tc:
        for _rep in range(reps):
            with tc.tile_pool(name="dram", bufs=1, space="DRAM") as dram:
                hs_bf = dram.tile([S, D, B_LOC], bf)
                _phase_a(nc, tc, T, hs_bf)
                _phase_b(nc, tc, T, hs_bf)

    nc.compile()
    return nc


def _prep_inputs(inputs):
    import ml_dtypes
    npf32 = np.float32
    npbf = ml_dtypes.bfloat16
    npf8 = ml_dtypes.float8_e4m3
    xs = np.stack([np.asarray(inputs["x1"], npf32),
                   np.asarray(inputs["x2"], npf32),
                   np.asarray(inputs["x3"], npf32)])  # (3, B, I)
    shared = {}
    for d in ("f", "b"):
        wih = np.asarray(inputs[f"W_ih_{d}"], npf32)
        whh = np.asarray(inputs[f"W_hh_{d}"], npf32)
        bih = np.asarray(inputs[f"b_ih_{d}"], npf32)
        bhh = np.asarray(inputs[f"b_hh_{d}"], npf32)
        wc = np.concatenate([wih.T, whh.T], axis=0)  # (2I, 3H)
        cols = []
        for j in range(HJT):
            for g in range(3):
                cols.append(wc[:, (g * H + j * P):(g * H + (j + 1) * P)])
        shared[f"wcat_{d}"] = np.ascontiguousarray(
            np.concatenate(cols, axis=1)).astype(npbf)
        bsum = bih + bhh
        shared[f"brz_{d}"] = np.ascontiguousarray(bsum[:2 * H, None])
        shared[f"negbz_{d}"] = np.ascontiguousarray(-bsum[H:2 * H, None])
        shared[f"bnih_{d}"] = np.ascontiguousarray(bih[2 * H:, None])
        shared[f"bnhh_{d}"] = np.ascontiguousarray(bhh[2 * H:, None])
    shared["wq8"] = np.ascontiguousarray(
        np.asarray(inputs["Wq"], npf32).T * W8_SCALE).astype(npf8)
    shared["wk8"] = np.ascontiguousarray(
        np.asarray(inputs["Wk"], npf32).T * W8_SCALE).astype(npf8)
    shared["wv"] = np.ascontiguousarray(
        np.asarray(inputs["Wv"], npf32).T).astype(npbf)
    weff = (np.asarray(inputs["W_out"], np.float64)
            @ np.asarray(inputs["Wo"], np.float64)).T
    shared["weff"] = np.ascontiguousarray(weff.astype(npf32)).astype(npbf)
    shared["bout"] = np.ascontiguousarray(
        np.asarray(inputs["b_out"], npf32)[None, :])

    in_maps = []
    for c in range(N_CORES):
        rows = slice(c * B_LOC, (c + 1) * B_LOC)
        m = dict(shared)
        m["xt"] = np.ascontiguousarray(
            xs[:, rows, :].transpose(0, 2, 1)).astype(npbf)
        in_maps.append(m)
    return in_maps


def _get_nc():
    if "nc" not in _CACHE:
        _CACHE["nc"] = build_nc()
    return _CACHE["nc"]


def kernel(**inputs):
    from concourse.bass_utils import run_bass_kernel_spmd

    nc = _get_nc()
    in_maps = _prep_inputs(inputs)
    res = run_bass_kernel_spmd(nc, in_maps, core_ids=list(range(N_CORES)))
    o = np.concatenate([res.results[c]["o_out"] for c in range(N_CORES)], axis=0)
    sm = np.concatenate([res.results[c]["sm_out"] for c in range(N_CORES)], axis=0)
    return o, sm


# revision 9
# speedup vs baseline: 1.3430x; 1.2331x over previous
"""HELMo encoder (bi-GRU over 3 steps + MHA + classifier) on 8 trn2 cores.

Data-parallel over batch (8192 -> 8 x 1024). Per core, one Bass/Tile kernel:
  A) fused GRU in bf16: input+hidden projections accumulate into f32 PSUM
     (k = [x; h_prev] vs W_cat = [W_ih.T; W_hh.T]); gates split across
     ACT (sigmoid/tanh), DVE (psum-side adds/muls) and Pool (SBUF-side
     combine); h stored bf16, feature-major.
  B) fused attention pass, one 128-row batch chunk at a time, all-SBUF:
     - Q/K projections as fp8e4 DoubleRow matmuls (2x PE rate); weights
       pre-scaled x16, hs x4 (exact powers of 2, fp8 normal range), with
       the compensation folded into the softmax exp scale.
     - V projection in bf16, W_v streamed per chunk in 512-col pieces.
     - per-head logits / softmax / (sum_tq w) combine on DVE+Pool.
     - W_o is folded into the classifier on the host
       (W_eff = W_out @ W_o), so ctx_sum goes straight to the 7-class
       head after a PE transpose; softmax on ACT/DVE.

Matmul accumulation is f32 PSUM throughout; rel err vs f32 reference
~2e-3, dominated by bf16 weight quantization.
"""

import sys

sys.path.insert(0, "/opt/trn_rl_repo")

import numpy as np

import concourse.bacc as bacc
import concourse.bass as bass
import concourse.mybir as mybir
import concourse.tile as tile
from concourse.masks import make_identity

dt = mybir.dt
AF = mybir.ActivationFunctionType
AX = mybir.AxisListType
PM = mybir.MatmulPerfMode

N_CORES = 8
B = 8192
B_LOC = B // N_CORES          # 1024
I = 1024
H = 1024
D = 2 * H                     # 2048
NH = 16
HD = 128
S = 3
C = 7
P = 128
HJT = H // P                  # 8 jtiles per gate
KC_D = D // P                 # 16
NCH = B_LOC // P              # 8 batch chunks in the attention pass

W8_SCALE = 16.0               # fp8 weight pre-scale (host side)
H8_SCALE = 4.0                # fp8 activation pre-scale (device side)
EXP_SCALE = 1.0 / (W8_SCALE * W8_SCALE * H8_SCALE * H8_SCALE * HD ** 0.5)
USE_DOUBLE_ROW = True

f32, bf, f8 = dt.float32, dt.bfloat16, dt.float8e4

_CACHE = {}


def _r3(ap, pat, **kw):
    return ap.rearrange(pat, **kw)


def _gru_cell(nc, pps, gpool, tpool, bias_t, j, bs, first,
              x_chunk, h_prev, h_new, wr, wz, wn):
    t_brz, t_nbz, t_bni, t_bnh = bias_t
    if first:
        pz = pps.tile([P, 512], f32, tag="pz")
        for c in range(HJT):
            nc.tensor.matmul(pz[:], wz[:, c, :], x_chunk(c)[:, bs],
                             start=(c == 0), stop=(c == HJT - 1))
        pgi = pps.tile([P, 512], f32, tag="pgi")
        for c in range(HJT):
            nc.tensor.matmul(pgi[:], wn[:, c, :], x_chunk(c)[:, bs],
                             start=(c == 0), stop=(c == HJT - 1))
        zc = gpool.tile([P, 512], f32, tag="z")
        nc.scalar.activation(zc[:], pz[:], AF.Sigmoid,
                             bias=t_nbz[:, j, :], scale=-1.0)
        n_sb = gpool.tile([P, 512], f32, tag="n")
        nc.scalar.activation(n_sb[:], pgi[:], AF.Tanh, bias=t_bni[:, j, :])
        nc.gpsimd.tensor_mul(h_new[:, j, bs], zc[:], n_sb[:])
        return

    nk = 2 * HJT

    def mm_acc(ptile, ws):
        for c in range(nk):
            rhs = (x_chunk(c)[:, bs] if c < HJT else h_prev[:, c - HJT, bs])
            nc.tensor.matmul(ptile[:], ws[:, c, :], rhs,
                             start=(c == 0), stop=(c == nk - 1))

    pr = pps.tile([P, 512], f32, tag="pr")
    mm_acc(pr, wr)
    pz = pps.tile([P, 512], f32, tag="pz")
    mm_acc(pz, wz)
    pgi = pps.tile([P, 512], f32, tag="pgi")
    for c in range(HJT):
        nc.tensor.matmul(pgi[:], wn[:, c, :], x_chunk(c)[:, bs],
                         start=(c == 0), stop=(c == HJT - 1))
    pgh = pps.tile([P, 512], f32, tag="pgh")
    for c in range(HJT, 2 * HJT):
        nc.tensor.matmul(pgh[:], wn[:, c, :], h_prev[:, c - HJT, bs],
                         start=(c == HJT), stop=(c == 2 * HJT - 1))
    r_sb = gpool.tile([P, 512], f32, tag="r")
    nc.scalar.activation(r_sb[:], pr[:], AF.Sigmoid, bias=t_brz[:, j, :])
    z_sb = gpool.tile([P, 512], f32, tag="z")
    nc.scalar.activation(z_sb[:], pz[:], AF.Sigmoid, bias=t_brz[:, HJT + j, :])
    t1 = tpool.tile([P, 512], f32, tag="tmp")
    nc.vector.tensor_scalar_add(t1[:], pgh[:], t_bnh[:, j, :])
    t2 = tpool.tile([P, 512], f32, tag="tmp")
    nc.vector.tensor_mul(t2[:], r_sb[:], t1[:])
    t3 = tpool.tile([P, 512], f32, tag="tmp")
    nc.vector.tensor_add(t3[:], pgi[:], t2[:])
    n_sb = gpool.tile([P, 512], f32, tag="n")
    nc.scalar.activation(n_sb[:], t3[:], AF.Tanh, bias=t_bni[:, j, :])
    t4 = tpool.tile([P, 512], bf, tag="tmpb")
    nc.gpsimd.tensor_sub(t4[:], h_prev[:, j, bs], n_sb[:])
    t5 = tpool.tile([P, 512], bf, tag="tmpb")
    nc.gpsimd.tensor_mul(t5[:], z_sb[:], t4[:])
    nc.gpsimd.tensor_add(h_new[:, j, bs], t5[:], n_sb[:])


def _phase_a(nc, tc, T, hs_bf):
    with (tc.tile_pool(name="ga_const", bufs=1) as cpool,
          tc.tile_pool(name="ga_x", bufs=3) as xpool,
          tc.tile_pool(name="ga_h", bufs=3) as hpool,
          tc.tile_pool(name="ga_w", bufs=2) as wpool,
          tc.tile_pool(name="ga_g", bufs=2) as gpool,
          tc.tile_pool(name="ga_t", bufs=3) as tpool,
          tc.tile_pool(name="ga_ps", bufs=2, space="PSUM") as pps):
        bias = {}
        for d in ("f", "b"):
            t_brz = cpool.tile([P, 2 * HJT, 1], f32, tag=f"brz{d}")
            nc.sync.dma_start(t_brz[:], _r3(T[f"brz_{d}"][:], "(c k) o -> k c o", k=P))
            t_nbz = cpool.tile([P, HJT, 1], f32, tag=f"nbz{d}")
            nc.sync.dma_start(t_nbz[:], _r3(T[f"negbz_{d}"][:], "(c k) o -> k c o", k=P))
            t_bni = cpool.tile([P, HJT, 1], f32, tag=f"bni{d}")
            nc.sync.dma_start(t_bni[:], _r3(T[f"bnih_{d}"][:], "(c k) o -> k c o", k=P))
            t_bnh = cpool.tile([P, HJT, 1], f32, tag=f"bnh{d}")
            nc.sync.dma_start(t_bnh[:], _r3(T[f"bnhh_{d}"][:], "(c k) o -> k c o", k=P))
            bias[d] = (t_brz, t_nbz, t_bni, t_bnh)

        order = [(0, "f", 0), (0, "b", 2), (1, "f", 1),
                 (1, "b", 1), (2, "f", 2), (2, "b", 0)]
        h_cur = {"f": None, "b": None}
        for step, d, t in order:
            first = step == 0
            x_halves = []
            for xh in range(2):
                xv = xpool.tile([P, HJT // 2, B_LOC], bf, tag="x",
                                name=f"x_{step}_{d}_{xh}")
                nc.sync.dma_start(
                    xv[:],
                    _r3(T["xt"][t][xh * (I // 2):(xh + 1) * (I // 2), :],
                        "(c k) b -> k c b", k=P))
                x_halves.append(xv)

            def x_chunk(c):
                return x_halves[c // (HJT // 2)][:, c % (HJT // 2), :]

            h_prev = h_cur[d]
            h_new = hpool.tile([P, HJT, B_LOC], bf, tag="h",
                               name=f"h_{step}_{d}")
            for j in range(HJT):
                # host pre-permutes wcat columns: per j the r/z/n gate
                # columns are adjacent -> one contiguous 384-col DMA
                nkc = HJT if first else 2 * HJT
                wj = wpool.tile([P, nkc, 3 * P], bf, tag="wj",
                                name=f"wj_{step}_{d}_{j}")
                nc.sync.dma_start(
                    wj[:],
                    _r3(T[f"wcat_{d}"][:nkc * P, j * 3 * P:(j + 1) * 3 * P],
                        "(c k) m -> k c m", k=P))
                wr = wj[:, :, 0:P]
                wz = wj[:, :, P:2 * P]
                wn = wj[:, :, 2 * P:3 * P]
                for bt in range(2):
                    bs = slice(bt * 512, (bt + 1) * 512)
                    _gru_cell(nc, pps, gpool, tpool, bias[d], j, bs, first,
                              x_chunk, h_prev, h_new, wr, wz, wn)
                    row = (0 if d == "f" else H) + j * P
                    nc.sync.dma_start(hs_bf[t, row:row + P, bs],
                                      h_new[:, j, bs])
            h_cur[d] = h_new


def _attn_chunk(nc, pools, consts, T, hs_bf, ci):
    (hpool, h8pool, wvpool, qkpool, vpool, appool, smpool, ctxpool, cmpool,
     epool, pps_qk, pps_v, pps_t, pps_c) = pools
    wq8_sb, wk8_sb, weff_sb, bout_sb, ident = consts
    bsl = slice(ci * P, (ci + 1) * P)
    hst, hst8 = [], []
    for t in range(S):
        ht = hpool.tile([P, KC_D, P], bf, tag=f"hst{t}", name=f"hst{t}_{ci}")
        nc.sync.dma_start(ht[:], _r3(hs_bf[t][:, bsl], "(c k) b -> k c b", k=P))
        h8 = h8pool.tile([P, KC_D, P], f8, tag=f"h8_{t}", name=f"h8_{t}_{ci}")
        nc.gpsimd.tensor_scalar_mul(h8[:], ht[:], H8_SCALE)
        hst.append(ht)
        hst8.append(h8)

    qt, kt = [], []
    for t in range(S):
        for wsb, lst, nm in ((wq8_sb, qt, "q"), (wk8_sb, kt, "k")):
            osb = qkpool.tile([P, D], bf, tag=f"{nm}{t}", name=f"{nm}{t}_{ci}")
            for do_ in range(4):
                po = pps_qk.tile([P, 512], f32, tag="pqk",
                                 name=f"p{nm}_{t}_{ci}_{do_}")
                if USE_DOUBLE_ROW:
                    for c in range(0, KC_D, 2):
                        nc.tensor.matmul(
                            po[:], hst8[t][:, c:c + 2, :],
                            wsb[:, c:c + 2, do_ * 512:(do_ + 1) * 512],
                            start=(c == 0), stop=(c == KC_D - 2),
                            perf_mode=PM.DoubleRow)
                else:
                    for c in range(KC_D):
                        nc.tensor.matmul(
                            po[:], hst8[t][:, c, :],
                            wsb[:, c, do_ * 512:(do_ + 1) * 512],
                            start=(c == 0), stop=(c == KC_D - 1))
                dsl = slice(do_ * 512, (do_ + 1) * 512)
                nc.scalar.copy(osb[:, dsl], po[:])
            lst.append(osb)

    vt = [vpool.tile([P, D], bf, tag=f"v{t}", name=f"v{t}_{ci}")
          for t in range(S)]
    for do_ in range(4):
        wvt = wvpool.tile([P, KC_D, 512], bf, tag="wv", name=f"wv_{ci}_{do_}")
        nc.sync.dma_start(
            wvt[:],
            _r3(T["wv"][:, do_ * 512:(do_ + 1) * 512], "(c k) m -> k c m", k=P))
        for t in range(S):
            pv = pps_v.tile([P, 512], f32, tag="pv", name=f"pv_{ci}_{do_}_{t}")
            for c in range(KC_D):
                nc.tensor.matmul(pv[:], hst[t][:, c, :], wvt[:, c, :],
                                 start=(c == 0), stop=(c == KC_D - 1))
            dsl = slice(do_ * 512, (do_ + 1) * 512)
            nc.scalar.copy(vt[t][:, dsl], pv[:])

    L = smpool.tile([P, NH, S, S], f32, tag="L")
    for tq in range(S):
        for tk in range(S):
            pr_ = appool.tile([P, D], bf, tag="prod", name=f"prod_{ci}_{tq}_{tk}")
            nc.vector.tensor_mul(pr_[:], qt[tq][:], kt[tk][:])
            nc.vector.reduce_sum(
                L[:, :, tq, tk], _r3(pr_[:], "p (h e) -> p h e", h=NH), axis=AX.X)
    # logits are O(0.1) after descaling: exp cannot overflow, skip max-sub
    E2 = smpool.tile([P, NH, S, S], f32, tag="E2")
    nc.scalar.activation(E2[:], L[:], AF.Exp, scale=EXP_SCALE)
    Ssum = smpool.tile([P, NH, S], f32, tag="Ssum")
    nc.vector.reduce_sum(Ssum[:], E2[:], axis=AX.X)
    Rs = smpool.tile([P, NH, S], f32, tag="Rs")
    nc.vector.reciprocal(Rs[:], Ssum[:])
    Wn = smpool.tile([P, NH, S, S], f32, tag="Wn")
    nc.vector.tensor_mul(Wn[:], E2[:],
                         Rs[:, :, :, None].broadcast_to([P, NH, S, S]))
    wsum = smpool.tile([P, NH, S], f32, tag="wsum")
    nc.vector.reduce_sum(wsum[:], _r3(Wn[:], "p h q k -> p h k q"), axis=AX.X)

    tm0 = ctxpool.tile([P, D], bf, tag="tm0", name=f"tm0_{ci}")
    nc.vector.tensor_mul(
        _r3(tm0[:], "p (h e) -> p h e", h=NH),
        _r3(vt[0][:], "p (h e) -> p h e", h=NH),
        wsum[:, :, 0][:, :, None].broadcast_to([P, NH, HD]))
    tm1 = ctxpool.tile([P, D], bf, tag="tm1", name=f"tm1_{ci}")
    nc.vector.tensor_mul(
        _r3(tm1[:], "p (h e) -> p h e", h=NH),
        _r3(vt[1][:], "p (h e) -> p h e", h=NH),
        wsum[:, :, 1][:, :, None].broadcast_to([P, NH, HD]))
    nc.vector.tensor_add(tm0[:], tm0[:], tm1[:])
    tm2 = ctxpool.tile([P, D], bf, tag="tm2", name=f"tm2_{ci}")
    nc.vector.tensor_mul(
        _r3(tm2[:], "p (h e) -> p h e", h=NH),
        _r3(vt[2][:], "p (h e) -> p h e", h=NH),
        wsum[:, :, 2][:, :, None].broadcast_to([P, NH, HD]))
    ctx = ctxpool.tile([P, D], bf, tag="ctx", name=f"ctx_{ci}")
    nc.vector.tensor_add(ctx[:], tm0[:], tm2[:])

    ctxm = cmpool.tile([P, KC_D, P], bf, tag="ctxm", name=f"ctxm_{ci}")
    for c in range(KC_D):
        pt = pps_t.tile([P, P], bf, tag="pt", name=f"pt_{ci}_{c}")
        nc.tensor.transpose(pt[:], ctx[:, c * P:(c + 1) * P], ident[:])
        nc.vector.tensor_copy(ctxm[:, c, :], pt[:])
    pf = pps_c.tile([P, C], f32, tag="pf", name=f"pf_{ci}")
    for c in range(KC_D):
        nc.tensor.matmul(pf[:], ctxm[:, c, :], weff_sb[:, c, :],
                         start=(c == 0), stop=(c == KC_D - 1))
    o_sb = epool.tile([P, C], f32, tag="osb", name=f"osb_{ci}")
    nc.vector.tensor_add(o_sb[:], pf[:], bout_sb[:])
    nc.sync.dma_start(T["o_out"][bsl, :], o_sb[:])
    mx = epool.tile([P, 1], f32, tag="mx")
    nc.vector.reduce_max(mx[:], o_sb[:], axis=AX.X)
    nmx = epool.tile([P, 1], f32, tag="nmx")
    nc.vector.tensor_scalar_mul(nmx[:], mx[:], -1.0)
    esb = epool.tile([P, C], f32, tag="esb")
    nc.scalar.activation(esb[:], o_sb[:], AF.Exp, bias=nmx[:])
    ssb = epool.tile([P, 1], f32, tag="ssb")
    nc.vector.reduce_sum(ssb[:], esb[:], axis=AX.X)
    rsb = epool.tile([P, 1], f32, tag="rsb")
    nc.vector.reciprocal(rsb[:], ssb[:])
    smsb = epool.tile([P, C], f32, tag="smsb")
    nc.vector.tensor_mul(smsb[:], esb[:], rsb[:].broadcast_to([P, C]))
    nc.sync.dma_start(T["sm_out"][bsl, :], smsb[:])


def _phase_b(nc, tc, T, hs_bf):
    with (tc.tile_pool(name="gb_c", bufs=1) as cpool,
          tc.tile_pool(name="gb_h", bufs=2) as hpool,
          tc.tile_pool(name="gb_h8", bufs=2) as h8pool,
          tc.tile_pool(name="gb_wv", bufs=2) as wvpool,
          tc.tile_pool(name="gb_qk", bufs=1) as qkpool,
          tc.tile_pool(name="gb_v", bufs=1) as vpool,
          tc.tile_pool(name="gb_ap", bufs=2) as appool,
          tc.tile_pool(name="gb_sm", bufs=1) as smpool,
          tc.tile_pool(name="gb_ctx", bufs=1) as ctxpool,
          tc.tile_pool(name="gb_cm", bufs=2) as cmpool,
          tc.tile_pool(name="gb_e", bufs=2) as epool,
          tc.tile_pool(name="gb_pqk", bufs=4, space="PSUM") as pps_qk,
          tc.tile_pool(name="gb_pv", bufs=2, space="PSUM") as pps_v,
          tc.tile_pool(name="gb_pt", bufs=1, space="PSUM") as pps_t,
          tc.tile_pool(name="gb_pc", bufs=1, space="PSUM") as pps_c):
        wq8_sb = cpool.tile([P, KC_D, D], f8, tag="wq8")
        nc.sync.dma_start(wq8_sb[:], _r3(T["wq8"][:], "(c k) n -> k c n", k=P))
        wk8_sb = cpool.tile([P, KC_D, D], f8, tag="wk8")
        nc.sync.dma_start(wk8_sb[:], _r3(T["wk8"][:], "(c k) n -> k c n", k=P))
        weff_sb = cpool.tile([P, KC_D, C], bf, tag="weff")
        nc.sync.dma_start(weff_sb[:], _r3(T["weff"][:], "(c k) n -> k c n", k=P))
        bout_sb = cpool.tile([P, C], f32, tag="bout")
        nc.sync.dma_start(bout_sb[:], T["bout"][:].to_broadcast([P, C]))
        ident = cpool.tile([P, P], bf, tag="ident")
        make_identity(nc, ident[:])

        pools = (hpool, h8pool, wvpool, qkpool, vpool, appool, smpool,
                 ctxpool, cmpool, epool, pps_qk, pps_v, pps_t, pps_c)
        consts = (wq8_sb, wk8_sb, weff_sb, bout_sb, ident)
        for ci in range(NCH):
            _attn_chunk(nc, pools, consts, T, hs_bf, ci)


def build_nc(reps=1, phases="ab"):
    nc = bacc.Bacc("TRN2", target_bir_lowering=False, debug=False,
                   num_devices=N_CORES, dynamic_dma_scratch_size=8192)

    T = {}
    T["xt"] = nc.dram_tensor("xt", [S, I, B_LOC], bf, kind="ExternalInput")
    for d in ("f", "b"):
        T[f"wcat_{d}"] = nc.dram_tensor(f"wcat_{d}", [2 * H, 3 * H], bf,
                                        kind="ExternalInput")
        T[f"brz_{d}"] = nc.dram_tensor(f"brz_{d}", [2 * H, 1], f32,
                                       kind="ExternalInput")
        T[f"negbz_{d}"] = nc.dram_tensor(f"negbz_{d}", [H, 1], f32,
                                         kind="ExternalInput")
        T[f"bnih_{d}"] = nc.dram_tensor(f"bnih_{d}", [H, 1], f32,
                                        kind="ExternalInput")
        T[f"bnhh_{d}"] = nc.dram_tensor(f"bnhh_{d}", [H, 1], f32,
                                        kind="ExternalInput")
    T["wq8"] = nc.dram_tensor("wq8", [D, D], f8, kind="ExternalInput")
    T["wk8"] = nc.dram_tensor("wk8", [D, D], f8, kind="ExternalInput")
    T["wv"] = nc.dram_tensor("wv", [D, D], bf, kind="ExternalInput")
    T["weff"] = nc.dram_tensor("weff", [D, C], bf, kind="ExternalInput")
    T["bout"] = nc.dram_tensor("bout", [1, C], f32, kind="ExternalInput")
    T["o_out"] = nc.dram_tensor("o_out", [B_LOC, C], f32, kind="ExternalOutput")
    T["sm_out"] = nc.dram_tensor("sm_out", [B_LOC, C], f32,
                                 kind="ExternalOutput")

    with tile.TileContext(nc) as tc:
        for _rep in range(reps):
            with tc.tile_pool(name="dram", bufs=1, space="DRAM") as dram:
                hs_bf = dram.tile([S, D, B_LOC], bf)
                if "a" in phases:
                    _phase_a(nc, tc, T, hs_bf)
                if "b" in phases:
                    _phase_b(nc, tc, T, hs_bf)

    nc.compile()
    return nc


def _prep_inputs(inputs):
    import ml_dtypes
    npf32 = np.float32
    npbf = ml_dtypes.bfloat16
    npf8 = ml_dtypes.float8_e4m3
    xs = np.stack([np.asarray(inputs["x1"], npf32),
                   np.asarray(inputs["x2"], npf32),
                   np.asarray(inputs["x3"], npf32)])  # (3, B, I)
    shared = {}
    for d in ("f", "b"):
        wih = np.asarray(inputs[f"W_ih_{d}"], npf32)
        whh = np.asarray(inputs[f"W_hh_{d}"], npf32)
        bih = np.asarray(inputs[f"b_ih_{d}"], npf32)
        bhh = np.asarray(inputs[f"b_hh_{d}"], npf32)
        wc = np.concatenate([wih.T, whh.T], axis=0)  # (2I, 3H)
        cols = []
        for j in range(HJT):
            for g in range(3):
                cols.append(wc[:, (g * H + j * P):(g * H + (j + 1) * P)])
        shared[f"wcat_{d}"] = np.ascontiguousarray(
            np.concatenate(cols, axis=1)).astype(npbf)
        bsum = bih + bhh
        shared[f"brz_{d}"] = np.ascontiguousarray(bsum[:2 * H, None])
        shared[f"negbz_{d}"] = np.ascontiguousarray(-bsum[H:2 * H, None])
        shared[f"bnih_{d}"] = np.ascontiguousarray(bih[2 * H:, None])
        shared[f"bnhh_{d}"] = np.ascontiguousarray(bhh[2 * H:, None])
    shared["wq8"] = np.ascontiguousarray(
        np.asarray(inputs["Wq"], npf32).T * W8_SCALE).astype(npf8)
    shared["wk8"] = np.ascontiguousarray(
        np.asarray(inputs["Wk"], npf32).T * W8_SCALE).astype(npf8)
    shared["wv"] = np.ascontiguousarray(
        np.asarray(inputs["Wv"], npf32).T).astype(npbf)
    weff = (np.asarray(inputs["W_out"], np.float64)
            @ np.asarray(inputs["Wo"], np.float64)).T
    shared["weff"] = np.ascontiguousarray(weff.astype(npf32)).astype(npbf)
    shared["bout"] = np.ascontiguousarray(
        np.asarray(inputs["b_out"], npf32)[None, :])

    in_maps = []
    for c in range(N_CORES):
        rows = slice(c * B_LOC, (c + 1) * B_LOC)
        m = dict(shared)
        m["xt"] = np.ascontiguousarray(
            xs[:, rows, :].transpose(0, 2, 1)).astype(npbf)
        in_maps.append(m)
    return in_maps


def _get_nc():
    if "nc" not in _CACHE:
        _CACHE["nc"] = build_nc()
    return _CACHE["nc"]


def kernel(**inputs):
    from concourse.bass_utils import run_bass_kernel_spmd

    nc = _get_nc()
    in_maps = _prep_inputs(inputs)
    res = run_bass_kernel_spmd(nc, in_maps, core_ids=list(range(N_CORES)))
    o = np.concatenate([res.results[c]["o_out"] for c in range(N_CORES)], axis=0)
    sm = np.concatenate([res.results[c]["sm_out"] for c in range(N_CORES)], axis=0)
    return o, sm
